# revision 27
# baseline (speedup 1.0000x reference)
"""Trainium2 Bass kernel for a single-query attention layer.

Reference computation (per batch b):
    q      = ht[b] @ W                      # (1, H)
    scores = q . h_0_t[b, t, :] over H      # (T,)
    alpha  = softmax(scores)                # (T,)
    ct[b]  = sum_t alpha[t] * h_0_t[b, t]   # (1, H)

Sharding: data-parallel over batch across 8 NeuronCores (8 batches per
core); the (H, H) weight is replicated.  No collectives.

Per-core dataflow (memory-bound; the 64 MiB h_0_t shard is read from
HBM exactly once):
  - stream h_0_t[b] in 1 MiB chunks into SBUF, natural [T-on-partitions,
    H-on-free] layout (the only DMA-efficient one for this DRAM layout)
  - scores: one fused multiply+reduce (tensor_tensor_reduce) per
    128-timestep tile on VectorE, against a partition-replicated q
  - softmax: free-axis max (DVE) -> partition max via PE transpose +
    free-axis max -> exp with fused bias and fused row-sum (ScalarE) ->
    partition sum via a ones-matmul (PE)
  - weighted sum: 64 TensorE matmuls per batch (contraction over T =
    partition axis, which the natural layout supports directly), fp32r
    at full PE rate, accumulated in PSUM
  - scale by 1/denominator during the PSUM->SBUF copy, DMA the row out

Batches are pipelined: the chunk pool holds 2 batches so batch b+1's
DMA+scores overlap batch b's softmax+weighted-sum.
"""

import sys

import numpy as np

_BASS_ROOT = "/opt/trn_rl_repo"
if _BASS_ROOT not in sys.path:
    sys.path.insert(0, _BASS_ROOT)

import concourse.bass as bass  # noqa: E402
from concourse import mybir  # noqa: E402
from concourse.bass_utils import run_bass_kernel_spmd  # noqa: E402
from concourse.tile import TileContext  # noqa: E402

B, T, H = 64, 8192, 256
N_CORES = 8
B_LOC = B // N_CORES  # batches per core
P = 128               # SBUF partitions
F32 = mybir.dt.float32
F32R = mybir.dt.float32r
I32 = mybir.dt.int32


def build_nc(b_loc=B_LOC, t=T, chunk_k=16, h_bufs=10, reps=1):
    """Build the per-core Bass graph.

    chunk_k: T-tiles (of 128 timesteps) per DMA chunk.
    h_bufs:  chunk-pool slots (h_bufs * chunk_k * 128 timesteps resident).
    reps:    unrolled repetitions of the whole batch loop (benchmarking
             aid — wall(reps=3) - wall(reps=1) = 2x the kernel time,
             cancelling dispatch overhead).
    """
    tpb = t // P              # T-tiles per batch
    n_chunks = tpb // chunk_k
    hc = H // P               # contraction chunks for the q matmul

    from concourse.bacc import Bacc

    nc = Bacc()
    h_d = nc.declare_dram_parameter("h_0_t", [b_loc, t, H], F32, isOutput=False)
    ht_d = nc.declare_dram_parameter("ht", [b_loc, 1, H], F32, isOutput=False)
    w_d = nc.declare_dram_parameter("weight", [H, H], F32, isOutput=False)
    out_d = nc.declare_dram_parameter("out", [b_loc, 1, H], F32, isOutput=True)

    with TileContext(nc) as tc:
        with (
            tc.tile_pool(name="const", bufs=1) as const_pool,
            tc.tile_pool(name="hbuf", bufs=h_bufs) as h_pool,
            tc.tile_pool(name="stats", bufs=2) as stats_pool,
            tc.tile_pool(name="scr", bufs=2) as scr_pool,
            tc.tile_pool(name="ctout", bufs=2) as out_pool,
            tc.tile_pool(name="ps", bufs=1, space="PSUM") as psum_pool,
            tc.tile_pool(name="qdram", bufs=1, space="DRAM") as dram_pool,
        ):
            # ---- constants ----
            ones_col = const_pool.tile([P, 1], F32, name="ones_col")
            nc.vector.memset(ones_col, 1.0)
            neg_ones_row = const_pool.tile([1, P], F32, name="neg_ones_row")
            nc.vector.memset(neg_ones_row, -1.0)
            ident_i = const_pool.tile([P, P], I32, name="ident_i")
            nc.gpsimd.iota(ident_i, pattern=[[-1, P]], base=0, channel_multiplier=1)
            ident = const_pool.tile([P, P], F32, name="ident")
            nc.vector.tensor_scalar(
                ident, ident_i, 0, None, op0=mybir.AluOpType.is_equal
            )

            # ---- q = ht @ W for all local batches (one-time setup) ----
            w_sb = const_pool.tile([P, hc, H], F32, name="w_sb")
            nc.sync.dma_start(
                out=w_sb, in_=w_d[:].rearrange("(c p) k -> p c k", p=P)
            )
            htT = const_pool.tile([P, hc, b_loc], F32, name="htT")
            for c in range(hc):
                nc.gpsimd.dma_start(
                    out=htT[:, c, :],
                    in_=ht_d[:, 0, c * P : (c + 1) * P].rearrange("b p -> p b"),
                )
            # dummy self-matmul absorbs the htT DMA wait so the q matmul
            # carries a single sync wait (PE LDWEIGHTS allows only one)
            dmy_ps = psum_pool.tile([b_loc, b_loc], F32, name="dmy_ps", tag="dmy")
            nc.tensor.matmul(
                dmy_ps, lhsT=htT[:, 0, :], rhs=htT[:, 0, :], start=True, stop=True
            )
            q_ps = psum_pool.tile([b_loc, H], F32, name="q_ps", tag="qps")
            for c in range(hc):
                nc.tensor.matmul(
                    q_ps, lhsT=htT[:, c, :], rhs=w_sb[:, c, :],
                    start=(c == 0), stop=(c == hc - 1),
                )
            q_sb = const_pool.tile([b_loc, H], F32, name="q_sb")
            nc.vector.tensor_copy(q_sb, q_ps)
            # replicate each batch's q across all 128 partitions (DRAM bounce)
            q_dram = dram_pool.tile([b_loc, H], F32, name="q_dram")
            nc.sync.dma_start(out=q_dram, in_=q_sb)
            q_rep = const_pool.tile([P, b_loc, H], F32, name="q_rep")
            q_bcast_src = bass.AP(
                tensor=q_dram.tensor, offset=q_dram.offset,
                ap=[[0, P], [H, b_loc], [1, H]],
            )
            nc.sync.dma_start(out=q_rep, in_=q_bcast_src)

            # ---- batch loop ----
            for b in [bb for _ in range(reps) for bb in range(b_loc)]:
                s_all = stats_pool.tile([P, tpb], F32, name="s_all", tag="s_all")
                chunks = []
                for c in range(n_chunks):
                    # declared fp32r so TensorE can consume it at full rate;
                    # the DVE scores path reads the same bits as fp32.
                    # Blocked T layout: partition p holds chunk_k CONSECUTIVE
                    # timesteps (contiguous chunk_k*1KB DRAM per partition ->
                    # large DMA descriptors). softmax + weighted sum are
                    # permutation-invariant over T, so the order change is
                    # harmless.
                    hch = h_pool.tile([P, chunk_k, H], F32R, name="hch", tag="hch")
                    src = h_d[b, c * chunk_k * P : (c + 1) * chunk_k * P, :]
                    nc.sync.dma_start(
                        out=hch,
                        in_=src.rearrange("(p k) j -> p k j", k=chunk_k).bitcast(F32R),
                    )
                    chunks.append(hch)
                    for k in range(chunk_k):
                        gk = c * chunk_k + k
                        vscr = scr_pool.tile([P, H], F32, name="vscr", tag="vscr")
                        nc.vector.scalar_tensor_tensor(
                            out=vscr, in0=hch[:, k, :].bitcast(F32),
                            scalar=1.0, in1=q_rep[:, b, :],
                            op0=mybir.AluOpType.mult, op1=mybir.AluOpType.mult,
                            accum_out=s_all[:, gk : gk + 1],
                        )

                # ---- softmax statistics ----
                m_col = stats_pool.tile([P, 1], F32, name="m_col", tag="m_col")
                nc.vector.reduce_max(m_col, s_all, axis=mybir.AxisListType.X)
                mT_ps = psum_pool.tile([1, P], F32, name="mT_ps", tag="mT")
                nc.tensor.transpose(mT_ps, m_col, ident)
                m_sb = stats_pool.tile([1, 1], F32, name="m_sb", tag="m_sb")
                nc.vector.reduce_max(m_sb, mT_ps, axis=mybir.AxisListType.X)
                # broadcast -max to all partitions via a C=1 matmul
                negm_ps = psum_pool.tile([P, 1], F32, name="negm_ps", tag="negm")
                nc.tensor.matmul(
                    negm_ps, lhsT=neg_ones_row, rhs=m_sb, start=True, stop=True
                )
                negm_sb = stats_pool.tile([P, 1], F32, name="negm_sb", tag="negm_sb")
                nc.vector.tensor_copy(negm_sb, negm_ps)
                p_all = stats_pool.tile([P, tpb], F32R, name="p_all", tag="p_all")
                l_col = stats_pool.tile([P, 1], F32, name="l_col", tag="l_col")
                nc.scalar.activation(
                    out=p_all, in_=s_all, func=mybir.ActivationFunctionType.Exp,
                    bias=negm_sb, scale=1.0, accum_out=l_col,
                )
                l_ps = psum_pool.tile([1, 1], F32, name="l_ps", tag="l")
                nc.tensor.matmul(
                    l_ps, lhsT=l_col, rhs=ones_col, start=True, stop=True
                )
                inv_l = stats_pool.tile([1, 1], F32, name="inv_l", tag="inv_l")
                nc.vector.reciprocal(inv_l, l_ps)

                # ---- weighted sum over T on TensorE ----
                ct_ps = psum_pool.tile([1, H], F32, name="ct_ps", tag="ct", bufs=2)
                for c in range(n_chunks):
                    for k in range(chunk_k):
                        gk = c * chunk_k + k
                        nc.tensor.matmul(
                            ct_ps, lhsT=p_all[:, gk : gk + 1],
                            rhs=chunks[c][:, k, :],
                            start=(gk == 0), stop=(gk == tpb - 1),
                        )
                ct_sb = out_pool.tile([1, H], F32, name="ct_sb", tag="ct_sb")
                nc.vector.tensor_scalar_mul(ct_sb, ct_ps, inv_l[0:1, 0:1])
                nc.sync.dma_start(out=out_d[b, :, :], in_=ct_sb)

    # Bacc.finalize() runs the lowering passes raw Bass lacks: matmul-wait
    # relocation, event-semaphore wait splitting (HW allows 1 wait/inst),
    # GPSIMD library loads, ACT table loads, and extended-ISA codegen.
    if not nc.is_finalized():
        nc.finalize()
    return nc


_nc_cache = None


def _get_nc():
    global _nc_cache
    if _nc_cache is None:
        _nc_cache = build_nc()
    return _nc_cache


def _run(inputs, trace=False, **kw):
    nc = _get_nc()
    ht = np.ascontiguousarray(np.asarray(inputs["ht"], dtype=np.float32))
    h0 = np.asarray(inputs["h_0_t"], dtype=np.float32)
    w = np.ascontiguousarray(np.asarray(inputs["weight"], dtype=np.float32))
    in_maps = []
    for i in range(N_CORES):
        sl = slice(i * B_LOC, (i + 1) * B_LOC)
        in_maps.append(
            {
                "h_0_t": np.ascontiguousarray(h0[sl]),
                "ht": np.ascontiguousarray(ht[sl]),
                "weight": w,
            }
        )
    res = run_bass_kernel_spmd(
        nc, in_maps, core_ids=list(range(N_CORES)), trace=trace, **kw
    )
    out = np.concatenate([r["out"] for r in res.results], axis=0)
    return out, res


def kernel(**inputs):
    out, _ = _run(inputs)
    return out


# ---------------------------------------------------------------------------
# Timing helper (used by test.py only; not part of the grading contract).
# Rebuilds the shard_map executable once so repeat calls reuse one compiled
# NEFF with device-resident inputs, then reports min wall-clock.
# ---------------------------------------------------------------------------


_nc_rep_cache = {}


def _get_exec(inputs, reps=1):
    """Build (once) and return a zero-arg callable running the reps-unrolled
    kernel on all 8 cores with device-resident inputs."""
    import jax
    from jax.experimental.shard_map import shard_map
    from jax.sharding import Mesh, NamedSharding, PartitionSpec

    from concourse import bass2jax

    if reps == 1:
        nc = _get_nc()
    else:
        if reps not in _nc_rep_cache:
            _nc_rep_cache[reps] = build_nc(reps=reps)
        nc = _nc_rep_cache[reps]
    bass2jax.install_neuronx_cc_hook()

    partition_name = (
        nc.partition_id_tensor.name if nc.partition_id_tensor else None
    )
    in_names, out_names, out_avals, zero_outs = [], [], [], []
    for alloc in nc.m.functions[0].allocations:
        if not isinstance(alloc, mybir.MemoryLocationSet):
            continue
        name = alloc.memorylocations[0].name
        if alloc.kind == "ExternalInput":
            if name != partition_name:
                in_names.append(name)
        elif alloc.kind == "ExternalOutput":
            out_names.append(name)
            shape = tuple(alloc.tensor_shape)
            dtype = mybir.dt.np(alloc.dtype)
            out_avals.append(jax.core.ShapedArray(shape, dtype))
            zero_outs.append(np.zeros(shape, dtype))
    n_params = len(in_names)
    n_outs = len(out_avals)
    all_names = list(in_names) + out_names
    if partition_name is not None:
        all_names.append(partition_name)

    def _body(*args):
        operands = list(args)
        if partition_name is not None:
            operands.append(bass2jax.partition_id_tensor())
        outs = bass2jax._bass_exec_p.bind(
            *operands,
            out_avals=tuple(out_avals),
            in_names=tuple(all_names),
            out_names=tuple(out_names),
            lowering_input_output_aliases=(),
            sim_require_finite=True,
            sim_require_nnan=True,
            nc=nc,
        )
        return tuple(outs)

    devices = jax.devices()[:N_CORES]
    mesh = Mesh(np.asarray(devices), ("core",))
    in_specs = (PartitionSpec("core"),) * (n_params + n_outs)
    out_specs = (PartitionSpec("core"),) * n_outs
    sharded = jax.jit(
        shard_map(
            _body, mesh=mesh, in_specs=in_specs, out_specs=out_specs,
            check_rep=False,
        ),
        keep_unused=True,
    )

    ht = np.ascontiguousarray(np.asarray(inputs["ht"], dtype=np.float32))
    h0 = np.ascontiguousarray(np.asarray(inputs["h_0_t"], dtype=np.float32))
    w = np.asarray(inputs["weight"], dtype=np.float32)
    per_core = {
        "h_0_t": h0,
        "ht": ht,
        "weight": np.concatenate([w[None]] * N_CORES, axis=0).reshape(
            N_CORES * w.shape[0], w.shape[1]
        ),
    }
    sh = NamedSharding(mesh, PartitionSpec("core"))
    xs = [jax.device_put(per_core[name], sh) for name in in_names]
    zs = [
        jax.device_put(
            np.zeros((N_CORES * z.shape[0], *z.shape[1:]), z.dtype), sh
        )
        for z in zero_outs
    ]

    def call():
        jax.block_until_ready(sharded(*xs, *zs))

    call()  # warm up (includes compile)
    return call


def time_kernel_pair(inputs, iters=60, reps_hi=3):
    """Interleaved slope timing: min(wall_hi) - min(wall_lo) over paired
    adjacent samples cancels axon dispatch overhead and its drift.
    Returns one kernel execution time in ns."""
    import time

    lo = _get_exec(inputs, reps=1)
    hi = _get_exec(inputs, reps=reps_hi)
    t_lo, t_hi = [], []
    for _ in range(iters):
        t0 = time.perf_counter()
        lo()
        t1 = time.perf_counter()
        hi()
        t2 = time.perf_counter()
        t_lo.append(t1 - t0)
        t_hi.append(t2 - t1)
    ns = (min(t_hi) - min(t_lo)) / (reps_hi - 1) * 1e9
    return ns, min(t_lo) * 1e9, min(t_hi) * 1e9


# revision 30
# speedup vs baseline: 33.2580x; 33.2580x over previous
"""Trainium2 Bass kernel for a single-query attention layer.

Reference computation (per batch b):
    q      = ht[b] @ W                      # (1, H)
    scores = q . h_0_t[b, t, :] over H      # (T,)
    alpha  = softmax(scores)                # (T,)
    ct[b]  = sum_t alpha[t] * h_0_t[b, t]   # (1, H)

Sharding: data-parallel over batch across 8 NeuronCores (8 batches per
core); the (H, H) weight is replicated.  No collectives.

Per-core dataflow (memory-bound; the 64 MiB h_0_t shard is read from
HBM exactly once):
  - stream h_0_t[b] in 1 MiB chunks into SBUF, natural [T-on-partitions,
    H-on-free] layout (the only DMA-efficient one for this DRAM layout)
  - scores: one fused multiply+reduce (tensor_tensor_reduce) per
    128-timestep tile on VectorE, against a partition-replicated q
  - softmax: free-axis max (DVE) -> partition max via PE transpose +
    free-axis max -> exp with fused bias and fused row-sum (ScalarE) ->
    partition sum via a ones-matmul (PE)
  - weighted sum: 64 TensorE matmuls per batch (contraction over T =
    partition axis, which the natural layout supports directly), fp32r
    at full PE rate, accumulated in PSUM
  - scale by 1/denominator during the PSUM->SBUF copy, DMA the row out

Batches are pipelined: the chunk pool holds 2 batches so batch b+1's
DMA+scores overlap batch b's softmax+weighted-sum.
"""

import sys

import numpy as np

_BASS_ROOT = "/opt/trn_rl_repo"
if _BASS_ROOT not in sys.path:
    sys.path.insert(0, _BASS_ROOT)

import concourse.bass as bass  # noqa: E402
from concourse import mybir  # noqa: E402
from concourse.bass_utils import run_bass_kernel_spmd  # noqa: E402
from concourse.tile import TileContext  # noqa: E402

B, T, H = 64, 8192, 256
N_CORES = 8
B_LOC = B // N_CORES  # batches per core
P = 128               # SBUF partitions
F32 = mybir.dt.float32
F32R = mybir.dt.float32r
I32 = mybir.dt.int32


def build_nc(b_loc=B_LOC, t=T, chunk_k=16, h_bufs=10, reps=1):
    """Build the per-core Bass graph.

    chunk_k: T-tiles (of 128 timesteps) per DMA chunk.
    h_bufs:  chunk-pool slots (h_bufs * chunk_k * 128 timesteps resident).
    reps:    unrolled repetitions of the whole batch loop (benchmarking
             aid — wall(reps=3) - wall(reps=1) = 2x the kernel time,
             cancelling dispatch overhead).
    """
    tpb = t // P              # T-tiles per batch
    n_chunks = tpb // chunk_k
    hc = H // P               # contraction chunks for the q matmul

    from concourse.bacc import Bacc

    nc = Bacc()
    h_d = nc.declare_dram_parameter("h_0_t", [b_loc, t, H], F32, isOutput=False)
    ht_d = nc.declare_dram_parameter("ht", [b_loc, 1, H], F32, isOutput=False)
    w_d = nc.declare_dram_parameter("weight", [H, H], F32, isOutput=False)
    out_d = nc.declare_dram_parameter("out", [b_loc, 1, H], F32, isOutput=True)

    with TileContext(nc) as tc:
        with (
            tc.tile_pool(name="const", bufs=1) as const_pool,
            tc.tile_pool(name="hbuf", bufs=h_bufs) as h_pool,
            tc.tile_pool(name="stats", bufs=2) as stats_pool,
            tc.tile_pool(name="scr", bufs=2) as scr_pool,
            tc.tile_pool(name="ctout", bufs=2) as out_pool,
            tc.tile_pool(name="ps", bufs=1, space="PSUM") as psum_pool,
            tc.tile_pool(name="qdram", bufs=1, space="DRAM") as dram_pool,
        ):
            # ---- constants ----
            ones_col = const_pool.tile([P, 1], F32, name="ones_col")
            nc.vector.memset(ones_col, 1.0)
            neg_ones_row = const_pool.tile([1, P], F32, name="neg_ones_row")
            nc.vector.memset(neg_ones_row, -1.0)
            ident_i = const_pool.tile([P, P], I32, name="ident_i")
            nc.gpsimd.iota(ident_i, pattern=[[-1, P]], base=0, channel_multiplier=1)
            ident = const_pool.tile([P, P], F32, name="ident")
            nc.vector.tensor_scalar(
                ident, ident_i, 0, None, op0=mybir.AluOpType.is_equal
            )

            # ---- q = ht @ W for all local batches (one-time setup) ----
            w_sb = const_pool.tile([P, hc, H], F32, name="w_sb")
            nc.sync.dma_start(
                out=w_sb, in_=w_d[:].rearrange("(c p) k -> p c k", p=P)
            )
            htT = const_pool.tile([P, hc, b_loc], F32, name="htT")
            for c in range(hc):
                nc.gpsimd.dma_start(
                    out=htT[:, c, :],
                    in_=ht_d[:, 0, c * P : (c + 1) * P].rearrange("b p -> p b"),
                )
            # dummy self-matmul absorbs the htT DMA wait so the q matmul
            # carries a single sync wait (PE LDWEIGHTS allows only one)
            dmy_ps = psum_pool.tile(
                [b_loc, b_loc], F32, name="dmy_ps", tag="dmy", bufs=2
            )
            nc.tensor.matmul(
                dmy_ps, lhsT=htT[:, 0, :], rhs=htT[:, 0, :], start=True, stop=True
            )
            q_ps = psum_pool.tile([b_loc, H], F32, name="q_ps", tag="qps")
            for c in range(hc):
                nc.tensor.matmul(
                    q_ps, lhsT=htT[:, c, :], rhs=w_sb[:, c, :],
                    start=(c == 0), stop=(c == hc - 1),
                )
            q_sb = const_pool.tile([b_loc, H], F32, name="q_sb")
            nc.vector.tensor_copy(q_sb, q_ps)
            # replicate each batch's q across all 128 partitions (DRAM bounce)
            q_dram = dram_pool.tile([b_loc, H], F32, name="q_dram")
            nc.sync.dma_start(out=q_dram, in_=q_sb)
            q_rep = const_pool.tile([P, b_loc, H], F32, name="q_rep")
            q_bcast_src = bass.AP(
                tensor=q_dram.tensor, offset=q_dram.offset,
                ap=[[0, P], [H, b_loc], [1, H]],
            )
            nc.sync.dma_start(out=q_rep, in_=q_bcast_src)

            # ---- batch loop ----
            for b in [bb for _ in range(reps) for bb in range(b_loc)]:
                s_all = stats_pool.tile([P, tpb], F32, name="s_all", tag="s_all")
                chunks = []
                for c in range(n_chunks):
                    # declared fp32r so TensorE can consume it at full rate;
                    # the DVE scores path reads the same bits as fp32.
                    # Blocked T layout: partition p holds chunk_k CONSECUTIVE
                    # timesteps (contiguous chunk_k*1KB DRAM per partition ->
                    # large DMA descriptors). softmax + weighted sum are
                    # permutation-invariant over T, so the order change is
                    # harmless.
                    hch = h_pool.tile([P, chunk_k, H], F32R, name="hch", tag="hch")
                    src = h_d[b, c * chunk_k * P : (c + 1) * chunk_k * P, :]
                    nc.sync.dma_start(
                        out=hch,
                        in_=src.rearrange("(p k) j -> p k j", k=chunk_k).bitcast(F32R),
                    )
                    chunks.append(hch)
                    for k in range(chunk_k):
                        gk = c * chunk_k + k
                        vscr = scr_pool.tile([P, H], F32, name="vscr", tag="vscr")
                        nc.vector.scalar_tensor_tensor(
                            out=vscr, in0=hch[:, k, :].bitcast(F32),
                            scalar=1.0, in1=q_rep[:, b, :],
                            op0=mybir.AluOpType.mult, op1=mybir.AluOpType.mult,
                            accum_out=s_all[:, gk : gk + 1],
                        )
                        if gk % 4 == 0:
                            # tiny dummy matmul threaded along the scores
                            # timeline (reads the column DVE just wrote) to
                            # keep the PE HAM clock-gate from re-throttling
                            # between weighted-sum bursts
                            warm_ps = psum_pool.tile(
                                [1, 1], F32, name="warm_ps", tag="dmy", bufs=2
                            )
                            nc.tensor.matmul(
                                warm_ps, lhsT=s_all[:, gk : gk + 1],
                                rhs=ones_col, start=True, stop=True,
                            )

                # ---- softmax statistics ----
                m_col = stats_pool.tile([P, 1], F32, name="m_col", tag="m_col")
                nc.vector.reduce_max(m_col, s_all, axis=mybir.AxisListType.X)
                mT_ps = psum_pool.tile([1, P], F32, name="mT_ps", tag="mT")
                nc.tensor.transpose(mT_ps, m_col, ident)
                m_sb = stats_pool.tile([1, 1], F32, name="m_sb", tag="m_sb")
                nc.vector.reduce_max(m_sb, mT_ps, axis=mybir.AxisListType.X)
                # broadcast -max to all partitions via a C=1 matmul
                negm_ps = psum_pool.tile([P, 1], F32, name="negm_ps", tag="negm")
                nc.tensor.matmul(
                    negm_ps, lhsT=neg_ones_row, rhs=m_sb, start=True, stop=True
                )
                negm_sb = stats_pool.tile([P, 1], F32, name="negm_sb", tag="negm_sb")
                nc.vector.tensor_copy(negm_sb, negm_ps)
                p_all = stats_pool.tile([P, tpb], F32R, name="p_all", tag="p_all")
                l_col = stats_pool.tile([P, 1], F32, name="l_col", tag="l_col")
                nc.scalar.activation(
                    out=p_all, in_=s_all, func=mybir.ActivationFunctionType.Exp,
                    bias=negm_sb, scale=1.0, accum_out=l_col,
                )
                l_ps = psum_pool.tile([1, 1], F32, name="l_ps", tag="l")
                nc.tensor.matmul(
                    l_ps, lhsT=l_col, rhs=ones_col, start=True, stop=True
                )
                inv_l = stats_pool.tile([1, 1], F32, name="inv_l", tag="inv_l")
                nc.vector.reciprocal(inv_l, l_ps)

                # ---- weighted sum over T on TensorE ----
                ct_ps = psum_pool.tile([1, H], F32, name="ct_ps", tag="ct", bufs=2)
                for c in range(n_chunks):
                    for k in range(chunk_k):
                        gk = c * chunk_k + k
                        nc.tensor.matmul(
                            ct_ps, lhsT=p_all[:, gk : gk + 1],
                            rhs=chunks[c][:, k, :],
                            start=(gk == 0), stop=(gk == tpb - 1),
                        )
                ct_sb = out_pool.tile([1, H], F32, name="ct_sb", tag="ct_sb")
                nc.vector.tensor_scalar_mul(ct_sb, ct_ps, inv_l[0:1, 0:1])
                nc.sync.dma_start(out=out_d[b, :, :], in_=ct_sb)

    # Bacc.finalize() runs the lowering passes raw Bass lacks: matmul-wait
    # relocation, event-semaphore wait splitting (HW allows 1 wait/inst),
    # GPSIMD library loads, ACT table loads, and extended-ISA codegen.
    if not nc.is_finalized():
        nc.finalize()
    return nc


_nc_cache = None


def _get_nc():
    global _nc_cache
    if _nc_cache is None:
        _nc_cache = build_nc()
    return _nc_cache


def _run(inputs, trace=False, **kw):
    nc = _get_nc()
    ht = np.ascontiguousarray(np.asarray(inputs["ht"], dtype=np.float32))
    h0 = np.asarray(inputs["h_0_t"], dtype=np.float32)
    w = np.ascontiguousarray(np.asarray(inputs["weight"], dtype=np.float32))
    in_maps = []
    for i in range(N_CORES):
        sl = slice(i * B_LOC, (i + 1) * B_LOC)
        in_maps.append(
            {
                "h_0_t": np.ascontiguousarray(h0[sl]),
                "ht": np.ascontiguousarray(ht[sl]),
                "weight": w,
            }
        )
    res = run_bass_kernel_spmd(
        nc, in_maps, core_ids=list(range(N_CORES)), trace=trace, **kw
    )
    out = np.concatenate([r["out"] for r in res.results], axis=0)
    return out, res


def kernel(**inputs):
    out, _ = _run(inputs)
    return out


# ---------------------------------------------------------------------------
# Timing helper (used by test.py only; not part of the grading contract).
# Rebuilds the shard_map executable once so repeat calls reuse one compiled
# NEFF with device-resident inputs, then reports min wall-clock.
# ---------------------------------------------------------------------------


_nc_rep_cache = {}


def _get_exec(inputs, reps=1):
    """Build (once) and return a zero-arg callable running the reps-unrolled
    kernel on all 8 cores with device-resident inputs."""
    import jax
    from jax.experimental.shard_map import shard_map
    from jax.sharding import Mesh, NamedSharding, PartitionSpec

    from concourse import bass2jax

    if reps == 1:
        nc = _get_nc()
    else:
        if reps not in _nc_rep_cache:
            _nc_rep_cache[reps] = build_nc(reps=reps)
        nc = _nc_rep_cache[reps]
    bass2jax.install_neuronx_cc_hook()

    partition_name = (
        nc.partition_id_tensor.name if nc.partition_id_tensor else None
    )
    in_names, out_names, out_avals, zero_outs = [], [], [], []
    for alloc in nc.m.functions[0].allocations:
        if not isinstance(alloc, mybir.MemoryLocationSet):
            continue
        name = alloc.memorylocations[0].name
        if alloc.kind == "ExternalInput":
            if name != partition_name:
                in_names.append(name)
        elif alloc.kind == "ExternalOutput":
            out_names.append(name)
            shape = tuple(alloc.tensor_shape)
            dtype = mybir.dt.np(alloc.dtype)
            out_avals.append(jax.core.ShapedArray(shape, dtype))
            zero_outs.append(np.zeros(shape, dtype))
    n_params = len(in_names)
    n_outs = len(out_avals)
    all_names = list(in_names) + out_names
    if partition_name is not None:
        all_names.append(partition_name)

    def _body(*args):
        operands = list(args)
        if partition_name is not None:
            operands.append(bass2jax.partition_id_tensor())
        outs = bass2jax._bass_exec_p.bind(
            *operands,
            out_avals=tuple(out_avals),
            in_names=tuple(all_names),
            out_names=tuple(out_names),
            lowering_input_output_aliases=(),
            sim_require_finite=True,
            sim_require_nnan=True,
            nc=nc,
        )
        return tuple(outs)

    devices = jax.devices()[:N_CORES]
    mesh = Mesh(np.asarray(devices), ("core",))
    in_specs = (PartitionSpec("core"),) * (n_params + n_outs)
    out_specs = (PartitionSpec("core"),) * n_outs
    sharded = jax.jit(
        shard_map(
            _body, mesh=mesh, in_specs=in_specs, out_specs=out_specs,
            check_rep=False,
        ),
        keep_unused=True,
    )

    ht = np.ascontiguousarray(np.asarray(inputs["ht"], dtype=np.float32))
    h0 = np.ascontiguousarray(np.asarray(inputs["h_0_t"], dtype=np.float32))
    w = np.asarray(inputs["weight"], dtype=np.float32)
    per_core = {
        "h_0_t": h0,
        "ht": ht,
        "weight": np.concatenate([w[None]] * N_CORES, axis=0).reshape(
            N_CORES * w.shape[0], w.shape[1]
        ),
    }
    sh = NamedSharding(mesh, PartitionSpec("core"))
    xs = [jax.device_put(per_core[name], sh) for name in in_names]
    zs = [
        jax.device_put(
            np.zeros((N_CORES * z.shape[0], *z.shape[1:]), z.dtype), sh
        )
        for z in zero_outs
    ]

    def call():
        jax.block_until_ready(sharded(*xs, *zs))

    call()  # warm up (includes compile)
    return call


def time_kernel_pair(inputs, iters=60, reps_hi=3, reps_lo=1):
    """Interleaved slope timing: min(wall_hi) - min(wall_lo) over paired
    adjacent samples cancels axon dispatch overhead and its drift.
    Returns one kernel execution time in ns."""
    import time

    lo = _get_exec(inputs, reps=reps_lo)
    hi = _get_exec(inputs, reps=reps_hi)
    t_lo, t_hi = [], []
    for _ in range(iters):
        t0 = time.perf_counter()
        lo()
        t1 = time.perf_counter()
        hi()
        t2 = time.perf_counter()
        t_lo.append(t1 - t0)
        t_hi.append(t2 - t1)
    ns = (min(t_hi) - min(t_lo)) / (reps_hi - reps_lo) * 1e9
    return ns, min(t_lo) * 1e9, min(t_hi) * 1e9
